# revision 5
# baseline (speedup 1.0000x reference)
"""AdaptiveHadamardTransform on 8 TRN2 NeuronCores.

y = scale * FHT_4096(x) + shift, x: (4, 4096, 4096) f32.

Algorithm: H_4096 = H_32 (x) H_128 (Sylvester Kronecker factorization).
Each 4096-row, viewed as X[i, k] (i in [0,32), k in [0,128)), transforms as
    y[i', k'] = sum_{i,k} H32[i, i'] * H128[k, k'] * X[i, k]

Per 8-row group (r = row quad, t in [0,4) packed on partitions):
  stage 1 (data stationary, 8 matmuls ap=128):
      p1[k, (u,(t',i'))] = sum_{(t,i)} A[(t,i), k] * blockdiag4(H32)
  stage 2 (H128 stationary, 2 matmuls ap=512):
      p2[k', (u,(t',i'))] = sum_k H128[k,k'] * s1[k, ...]
  shift fold (rank-32 accumulate into the same PSUM, 2 matmuls ap=512):
      p2 += Tmat[i', k']  with Tmat = 64*shift2d/scale2d
  affine (1 DVE op): ot = p2 * (scale2d[i',k']/64)  ->  scale*FHT + shift.

Everything runs in bf16 (tolerance 2e-2; bf16 keeps rel err ~4e-3):
matmuls at 1 cycle/row, HBM traffic halved. The host pre-packs x into the
per-core tile layout [128(t,i), 512 r, 128 k] bf16 so every DMA is
contiguous per partition, and unpacks the [128 k', 512 r, 128 (t',i')]
bf16 output back to fp32.

Engine assignment per group: SP issues input DMA (one per 2 groups),
PE does all matmuls incl. the shift, ACT drains PSUM1 -> SBUF bf16,
DVE applies scale, GpSimd (SWDGE) issues output DMA (one per 2 groups).

Sharding: data-parallel over the 16384 rows -> 2048 rows per core;
scale/shift folded into per-tile constants, replicated to all cores.
"""

import sys

sys.path.insert(0, "/opt/trn_rl_repo")

import numpy as np
import ml_dtypes

BF16 = ml_dtypes.bfloat16

SIZE = 4096
N_CORES = 8
ROWS = 16384  # 4 * 4096
ROWS_PER_CORE = ROWS // N_CORES  # 2048
R_VALS = ROWS_PER_CORE // 4  # 512 "r" values (4 rows each)
GROUPS = R_VALS // 8  # 64 groups of 8 r (32 rows) each

_CACHE = {}


def _sylvester(m: int) -> np.ndarray:
    H = np.array([[1.0]], dtype=np.float32)
    for _ in range(m):
        H = np.block([[H, H], [H, -H]]).astype(np.float32)
    return H


def _build_nc():
    import concourse.mybir as mybir
    from concourse import bacc, tile

    f32 = mybir.dt.float32
    bf16 = mybir.dt.bfloat16
    nc = bacc.Bacc("TRN2", target_bir_lowering=False, debug=False, num_devices=N_CORES)

    # Pre-packed input: [p=(t,i), r, k] with p = t*32 + i, element = row
    # (4r+t), column i*128+k of the core's 2048x4096 slab.
    x = nc.dram_tensor("x", [128, R_VALS, 128], bf16, kind="ExternalInput").ap()
    hbd4 = nc.dram_tensor("hbd4", [128, 128], bf16, kind="ExternalInput").ap()
    h128 = nc.dram_tensor("h128", [128, 128], bf16, kind="ExternalInput").ap()
    st2 = nc.dram_tensor("st2", [128, 1024], f32, kind="ExternalInput").ap()
    tmat = nc.dram_tensor("tmat", [32, 128], bf16, kind="ExternalInput").ap()
    ind = nc.dram_tensor("ind", [32, 1024], bf16, kind="ExternalInput").ap()
    # Output: [k', r, (t',i')]
    out = nc.dram_tensor("out", [128, R_VALS, 128], bf16, kind="ExternalOutput").ap()

    with tile.TileContext(nc) as tc:
        with (
            tc.tile_pool(name="consts", bufs=1) as cpool,
            tc.tile_pool(name="a", bufs=3) as apool,
            tc.tile_pool(name="s1", bufs=3) as spool,
            tc.tile_pool(name="ot", bufs=3) as opool,
            tc.tile_pool(name="ps1", bufs=2, space="PSUM") as ppool1,
            tc.tile_pool(name="ps2", bufs=2, space="PSUM") as ppool2,
        ):
            hbd_t = cpool.tile([128, 128], bf16)
            nc.scalar.dma_start(hbd_t[:], hbd4[:])
            h128_t = cpool.tile([128, 128], bf16)
            nc.scalar.dma_start(h128_t[:], h128[:])
            st2_t = cpool.tile([128, 1024], f32)
            nc.scalar.dma_start(st2_t[:], st2[:])
            tmat_t = cpool.tile([32, 128], bf16)
            nc.scalar.dma_start(tmat_t[:], tmat[:])
            ind_t = cpool.tile([32, 1024], bf16)
            nc.scalar.dma_start(ind_t[:], ind[:])

            obig = [None]  # current [128, 16, 128] output staging tile

            def stage2(s1, g):
                """Stage-2 + shift matmuls, scale, and (odd g) out-DMA."""
                p2 = ppool2.tile([128, 1024], f32)
                for h in range(2):
                    sl = slice(h * 512, (h + 1) * 512)
                    nc.tensor.matmul(
                        p2[:, sl], h128_t[:], s1[:, sl], start=True, stop=False
                    )
                    nc.tensor.matmul(
                        p2[:, sl], tmat_t[:], ind_t[:, sl], start=False, stop=True
                    )
                if g % 2 == 0:
                    obig[0] = opool.tile([128, 16, 128], bf16, name="ob")
                off = (g % 2) * 8
                ob = obig[0]
                otf = ob[:, off : off + 8, :].rearrange("p r k -> p (r k)")
                nc.vector.tensor_mul(otf, p2[:], st2_t[:])
                if g % 2 == 1:
                    r0 = (g - 1) * 8
                    nc.gpsimd.dma_start(out[:, r0 : r0 + 16, :], ob[:])

            pend = None  # (s1_tile, g)
            cur_a = None
            for g in range(GROUPS):
                if g % 2 == 0:
                    cur_a = apool.tile([128, 16, 128], bf16)
                    r0 = g * 8
                    if g == 0:
                        # fine-grained first loads: get the first rows in
                        # flight quickly so the PE starts early
                        for uu in range(4):
                            nc.sync.dma_start(
                                cur_a[:, uu * 4 : (uu + 1) * 4, :],
                                x[:, r0 + uu * 4 : r0 + (uu + 1) * 4, :],
                            )
                    else:
                        nc.sync.dma_start(cur_a[:], x[:, r0 : r0 + 16, :])
                half = (g % 2) * 8
                p1 = ppool1.tile([128, 1024], f32)
                for u in range(8):
                    nc.tensor.matmul(
                        p1[:, u * 128 : (u + 1) * 128],
                        cur_a[:, half + u, :],
                        hbd_t[:],
                        start=True,
                        stop=True,
                    )
                s1 = spool.tile([128, 1024], bf16)
                nc.scalar.copy(s1[:], p1[:])
                if pend is not None:
                    stage2(*pend)
                pend = (s1, g)
            stage2(*pend)
    nc.compile()
    return nc


def _get_nc():
    if "nc" not in _CACHE:
        _CACHE["nc"] = _build_nc()
    return _CACHE["nc"]


def _make_const_tiles(scale: np.ndarray, shift: np.ndarray):
    H32 = _sylvester(5)
    H128 = _sylvester(7)
    hbd4 = np.zeros((128, 128), dtype=np.float32)
    for t in range(4):
        hbd4[t * 32 : (t + 1) * 32, t * 32 : (t + 1) * 32] = H32
    s2d = scale.astype(np.float32).reshape(32, 128)  # [i', k']
    b2d = shift.astype(np.float32).reshape(32, 128)
    cols = np.arange(1024)
    # st2[k', (u,p')] = scale2d[p'%32, k']/64
    st2 = np.ascontiguousarray((s2d / 64.0)[cols % 32, :].T)
    # tmat[c, k'] = 64*shift2d[c,k']/scale2d[c,k']
    tmat = np.ascontiguousarray(64.0 * b2d / s2d).astype(BF16)
    # ind[c, (u,p')] = 1 if p'%32 == c
    ind = (cols[None, :] % 32 == np.arange(32)[:, None]).astype(BF16)
    return hbd4.astype(BF16), H128.astype(BF16), st2, tmat, np.ascontiguousarray(ind)


def _pack_core(xc16: np.ndarray) -> np.ndarray:
    """[2048, 4096] bf16 -> [128 (t,i), 512 r, 128 k] bf16 (contiguous)."""
    v = xc16.reshape(R_VALS, 4, 32, 128)  # r, t, i, k
    return np.ascontiguousarray(v.transpose(1, 2, 0, 3)).reshape(128, R_VALS, 128)


def _unpack_core(oc: np.ndarray) -> np.ndarray:
    """[128 k', 512 r, 128 (t',i')] bf16 -> [2048, 4096] f32."""
    v = oc.reshape(128, R_VALS, 4, 32).transpose(1, 2, 3, 0)  # r, t', i', k'
    return v.reshape(ROWS_PER_CORE, SIZE).astype(np.float32)


def kernel(x: np.ndarray, scale: np.ndarray, shift: np.ndarray) -> np.ndarray:
    from concourse.bass_utils import run_bass_kernel_spmd

    x = np.asarray(x)
    scale = np.asarray(scale)
    shift = np.asarray(shift)
    nc = _get_nc()
    xf = x.reshape(ROWS, SIZE).astype(BF16)
    hbd4, H128, st2, tmat, ind = _make_const_tiles(scale, shift)

    in_maps = []
    for c in range(N_CORES):
        in_maps.append(
            {
                "x": _pack_core(xf[c * ROWS_PER_CORE : (c + 1) * ROWS_PER_CORE]),
                "hbd4": hbd4,
                "h128": H128,
                "st2": st2,
                "tmat": tmat,
                "ind": ind,
            }
        )
    res = run_bass_kernel_spmd(nc, in_maps, core_ids=list(range(N_CORES)))
    out = np.concatenate(
        [_unpack_core(res.results[c]["out"]) for c in range(N_CORES)], axis=0
    )
    return out.reshape(x.shape)


# revision 7
# speedup vs baseline: 1.7846x; 1.7846x over previous
"""AdaptiveHadamardTransform on 8 TRN2 NeuronCores.

y = scale * FHT_4096(x) + shift, x: (4, 4096, 4096) f32.

Algorithm: H_4096 = H_32 (x) H_128 (Sylvester Kronecker factorization).
Each 4096-row, viewed as X[i, k] (i in [0,32), k in [0,128)), transforms as
    y[i', k'] = sum_{i,k} H32[i, i'] * H128[k, k'] * X[i, k]

Per 8-row group (r = row quad, t in [0,4) packed on partitions):
  stage 1 (data stationary, 8 matmuls ap=128):
      p1[k, (u,(t',i'))] = sum_{(t,i)} A[(t,i), k] * blockdiag4(H32)
  stage 2 (H128 stationary, 1 matmul ap=1024):
      p2[k', (u,(t',i'))] = sum_k H128[k,k'] * s1[k, ...]
  affine (1 DVE op): ot = p2 * (scale2d[i',k']/64).

The shift is folded into the INPUT on the host: adding the constant row
c = H4096 @ (64*shift/scale) / 4096 to every row of x makes the device's
Hadamard deliver the shift exactly (H(x+c) = Hx + 64*shift/scale, then
* scale/64 = scale*FHT(x) + shift). This costs nothing on device and adds
no error beyond the bf16 input rounding that happens anyway.

Everything runs in bf16 (tolerance 2e-2; measured rel err ~3e-3):
matmuls at 1 cycle/row, HBM traffic halved. The host pre-packs x into the
per-core tile layout [128(t,i), 512 r, 128 k] bf16 so every DMA is
contiguous per partition, and unpacks the [128 k', 512 r, 128 (t',i')]
bf16 output back to fp32.

Engine assignment per group: SP issues input DMA (one per 2 groups),
PE does all matmuls, ACT drains PSUM1 -> SBUF bf16, DVE applies scale,
GpSimd (SWDGE) issues output DMA (one per 2 groups).

Sharding: data-parallel over the 16384 rows -> 2048 rows per core;
scale/shift folded into per-tile constants, replicated to all cores.
"""

import sys

sys.path.insert(0, "/opt/trn_rl_repo")

import numpy as np
import ml_dtypes

BF16 = ml_dtypes.bfloat16

SIZE = 4096
N_CORES = 8
ROWS = 16384  # 4 * 4096
ROWS_PER_CORE = ROWS // N_CORES  # 2048
R_VALS = ROWS_PER_CORE // 4  # 512 "r" values (4 rows each)
GROUPS = R_VALS // 8  # 64 groups of 8 r (32 rows) each

_CACHE = {}


def _sylvester(m: int) -> np.ndarray:
    H = np.array([[1.0]], dtype=np.float32)
    for _ in range(m):
        H = np.block([[H, H], [H, -H]]).astype(np.float32)
    return H


def _build_nc():
    import concourse.mybir as mybir
    from concourse import bacc, tile

    f32 = mybir.dt.float32
    bf16 = mybir.dt.bfloat16
    nc = bacc.Bacc("TRN2", target_bir_lowering=False, debug=False, num_devices=N_CORES)

    # Pre-packed input: [p=(t,i), r, k] with p = t*32 + i, element = row
    # (4r+t), column i*128+k of the core's 2048x4096 slab.
    x = nc.dram_tensor("x", [128, R_VALS, 128], bf16, kind="ExternalInput").ap()
    hbd4 = nc.dram_tensor("hbd4", [128, 128], bf16, kind="ExternalInput").ap()
    h128 = nc.dram_tensor("h128", [128, 128], bf16, kind="ExternalInput").ap()
    st2 = nc.dram_tensor("st2", [128, 1024], f32, kind="ExternalInput").ap()
    # Output: [k', r, (t',i')]
    out = nc.dram_tensor("out", [128, R_VALS, 128], bf16, kind="ExternalOutput").ap()

    with tile.TileContext(nc) as tc:
        with (
            tc.tile_pool(name="consts", bufs=1) as cpool,
            tc.tile_pool(name="a", bufs=3) as apool,
            tc.tile_pool(name="s1", bufs=3) as spool,
            tc.tile_pool(name="ot", bufs=3) as opool,
            tc.tile_pool(name="ps1", bufs=2, space="PSUM") as ppool1,
            tc.tile_pool(name="ps2", bufs=2, space="PSUM") as ppool2,
        ):
            hbd_t = cpool.tile([128, 128], bf16)
            nc.scalar.dma_start(hbd_t[:], hbd4[:])
            h128_t = cpool.tile([128, 128], bf16)
            nc.scalar.dma_start(h128_t[:], h128[:])
            st2_t = cpool.tile([128, 1024], f32)
            nc.scalar.dma_start(st2_t[:], st2[:])

            obig = [None]  # current [128, 16, 128] output staging tile

            def stage2(s1, g):
                """Stage-2 matmul, scale, and (odd g) out-DMA."""
                p2 = ppool2.tile([128, 1024], f32)
                for h in range(2):
                    sl = slice(h * 512, (h + 1) * 512)
                    nc.tensor.matmul(
                        p2[:, sl], h128_t[:], s1[:, sl], start=True, stop=True
                    )
                if g % 2 == 0:
                    obig[0] = opool.tile([128, 16, 128], bf16, name="ob")
                off = (g % 2) * 8
                ob = obig[0]
                otf = ob[:, off : off + 8, :].rearrange("p r k -> p (r k)")
                nc.vector.tensor_mul(otf, p2[:], st2_t[:])
                if g % 2 == 1:
                    r0 = (g - 1) * 8
                    nc.gpsimd.dma_start(out[:, r0 : r0 + 16, :], ob[:])

            pend = None  # (s1_tile, g)
            cur_a = None
            for g in range(GROUPS):
                if g % 2 == 0:
                    cur_a = apool.tile([128, 16, 128], bf16)
                    r0 = g * 8
                    if g == 0:
                        # fine-grained first loads: get the first rows in
                        # flight quickly so the PE starts early
                        for uu in range(4):
                            nc.sync.dma_start(
                                cur_a[:, uu * 4 : (uu + 1) * 4, :],
                                x[:, r0 + uu * 4 : r0 + (uu + 1) * 4, :],
                            )
                    else:
                        nc.sync.dma_start(cur_a[:], x[:, r0 : r0 + 16, :])
                half = (g % 2) * 8
                p1 = ppool1.tile([128, 1024], f32)
                for u in range(8):
                    nc.tensor.matmul(
                        p1[:, u * 128 : (u + 1) * 128],
                        cur_a[:, half + u, :],
                        hbd_t[:],
                        start=True,
                        stop=True,
                    )
                s1 = spool.tile([128, 1024], bf16)
                nc.scalar.copy(s1[:], p1[:])
                if pend is not None:
                    stage2(*pend)
                pend = (s1, g)
            stage2(*pend)
    nc.compile()
    return nc


def _get_nc():
    if "nc" not in _CACHE:
        _CACHE["nc"] = _build_nc()
    return _CACHE["nc"]


def _make_const_tiles(scale: np.ndarray, shift: np.ndarray):
    H32 = _sylvester(5)
    H128 = _sylvester(7)
    hbd4 = np.zeros((128, 128), dtype=np.float32)
    for t in range(4):
        hbd4[t * 32 : (t + 1) * 32, t * 32 : (t + 1) * 32] = H32
    s2d = scale.astype(np.float32).reshape(32, 128)  # [i', k']
    b2d = shift.astype(np.float32).reshape(32, 128)
    cols = np.arange(1024)
    # st2[k', (u,p')] = scale2d[p'%32, k']/64
    st2 = np.ascontiguousarray((s2d / 64.0)[cols % 32, :].T)
    # shift preseed row: c = H4096 @ (64*shift/scale) / 4096, as [i, k]
    c2d = (H32 @ (64.0 * b2d / s2d) @ H128) / 4096.0
    c_row = c2d.reshape(SIZE)
    return hbd4.astype(BF16), H128.astype(BF16), st2, c_row


def _pack_core(xc16: np.ndarray) -> np.ndarray:
    """[2048, 4096] bf16 -> [128 (t,i), 512 r, 128 k] bf16 (contiguous)."""
    v = xc16.reshape(R_VALS, 4, 32, 128)  # r, t, i, k
    return np.ascontiguousarray(v.transpose(1, 2, 0, 3)).reshape(128, R_VALS, 128)


def _unpack_core(oc: np.ndarray) -> np.ndarray:
    """[128 k', 512 r, 128 (t',i')] bf16 -> [2048, 4096] f32."""
    v = oc.reshape(128, R_VALS, 4, 32).transpose(1, 2, 3, 0)  # r, t', i', k'
    return v.reshape(ROWS_PER_CORE, SIZE).astype(np.float32)


def kernel(x: np.ndarray, scale: np.ndarray, shift: np.ndarray) -> np.ndarray:
    from concourse.bass_utils import run_bass_kernel_spmd

    x = np.asarray(x)
    scale = np.asarray(scale)
    shift = np.asarray(shift)
    nc = _get_nc()
    hbd4, H128, st2, c_row = _make_const_tiles(scale, shift)
    xf = (x.reshape(ROWS, SIZE) + c_row[None, :]).astype(BF16)

    in_maps = []
    for c in range(N_CORES):
        in_maps.append(
            {
                "x": _pack_core(xf[c * ROWS_PER_CORE : (c + 1) * ROWS_PER_CORE]),
                "hbd4": hbd4,
                "h128": H128,
                "st2": st2,
            }
        )
    res = run_bass_kernel_spmd(nc, in_maps, core_ids=list(range(N_CORES)))
    out = np.concatenate(
        [_unpack_core(res.results[c]["out"]) for c in range(N_CORES)], axis=0
    )
    return out.reshape(x.shape)
